# revision 15
# baseline (speedup 1.0000x reference)
"""DNC read-head kernel for Trainium2, data-parallel over batch on 8 NeuronCores.

Problem shapes (hardcoded): B=32, N=1024 memory slots, M=128 word size, H=2 heads.
Per core: 4 batches. Per batch the heavy work is two matvecs with L [1024,1024]:
f = L @ w on DVE via fused scalar_tensor_tensor (multiply + free-dim accumulate),
b = L^T @ w on the TensorEngine (float32r, accumulating over row blocks in PSUM),
plus cosine content addressing over memory [1024,128] and the read-mode combine.

The per-batch work is software-pipelined: the small serial softmax/combine tail
of batch i is emitted between the heavy streaming phases of batches i and i+1 so
it never head-of-line-blocks the DVE queue.

Layout convention: length-1024 vectors live as [128, 8] SBUF tiles where
element n = rb*128 + p sits at [p, rb].
"""

import numpy as np

B, N, M, H = 32, 1024, 128, 2
EPS = 1e-8
NCORES = 8
BPC = B // NCORES  # batches per core
NB = N // 128      # row/col blocks of L
NSCAL = 4          # beta/knorm, rm0, rm1, rm2

MM_F32R = True  # float32r matmuls: 1 cyc/row vs 4 for fp32, ~2e-5 rel err

_CACHE = {}


def _build_nc():
    import concourse.bacc as bacc
    import concourse.mybir as mybir
    from concourse.tile import TileContext
    from concourse.masks import make_identity

    f32 = mybir.dt.float32
    A = mybir.AluOpType
    AF = mybir.ActivationFunctionType

    nc = bacc.Bacc(
        "TRN2", target_bir_lowering=False, debug=False, num_devices=NCORES
    )
    mmdt = mybir.dt.float32r if MM_F32R else f32
    L_d = nc.declare_dram_parameter("L", [BPC, N, N], mmdt, isOutput=False)
    mem_d = nc.declare_dram_parameter("mem", [BPC, N, M], f32, isOutput=False)
    wbb_d = nc.declare_dram_parameter("wbb", [BPC, 128, N], f32, isOutput=False)
    wcol_d = nc.declare_dram_parameter("wcol", [128, BPC * NB], mmdt, isOutput=False)
    kb_d = nc.declare_dram_parameter("kb", [BPC, 128, M], f32, isOutput=False)
    scal_d = nc.declare_dram_parameter("scal", [128, NSCAL * BPC], f32, isOutput=False)
    Wout_d = nc.declare_dram_parameter("W_out", [BPC * NB, 128], f32, isOutput=True)
    mc_d = nc.declare_dram_parameter("mc_out", [1, BPC * M], f32, isOutput=True)

    with TileContext(nc) as tc:
        with (
            tc.tile_pool(name="singles", bufs=1) as singles,
            tc.tile_pool(name="lp", bufs=6) as lp,
            tc.tile_pool(name="mp", bufs=2) as mp,
            tc.tile_pool(name="scr", bufs=2) as scrp,
            tc.tile_pool(name="small", bufs=2) as smallp,
            tc.tile_pool(name="ps_b", bufs=1, space="PSUM") as ps_b,
            tc.tile_pool(name="ps_c", bufs=1, space="PSUM") as ps_c,
            tc.tile_pool(name="ps_d", bufs=2, space="PSUM") as ps_d,
        ):
            ident = singles.tile([128, 128], f32)
            make_identity(nc, ident[:])
            one11 = singles.tile([1, 1], f32)
            nc.gpsimd.memset(one11[:], 1.0)
            ones128 = singles.tile([128, 128], f32)
            nc.gpsimd.memset(ones128[:], 1.0)
            scal = singles.tile([128, NSCAL * BPC], f32)
            nc.sync.dma_start(out=scal[:], in_=scal_d[:])
            wcol = singles.tile([128, BPC * NB], mmdt)
            nc.sync.dma_start(out=wcol[:], in_=wcol_d[:])
            W_all = singles.tile([128, NB * BPC], f32)
            mc_all = singles.tile([1, BPC * M], f32)

            # per-batch state carried from the heavy phase to the tail
            state = {}

            def heavy(bi):
                sb = NSCAL * bi
                rm0_c = scal[:, sb + 1 : sb + 2]
                rm2_c = scal[:, sb + 3 : sb + 4]

                wb = smallp.tile([128, N], f32, tag="wb")
                nc.sync.dma_start(out=wb[:], in_=wbb_d[bi])
                kb = smallp.tile([128, M], f32, tag="kb")
                nc.sync.dma_start(out=kb[:], in_=kb_d[bi])

                # f = L @ w (DVE fused multiply+reduce), b = L^T @ w (PE)
                f_col = smallp.tile([128, NB], f32, tag="fcol")
                pb_lo = ps_b.tile([1, 512], f32, tag="pblo")
                pb_hi = ps_b.tile([1, 512], f32, tag="pbhi")
                for rb in range(NB):
                    Lt = lp.tile([128, N], mmdt, tag="L")
                    nc.sync.dma_start(
                        out=Lt[:], in_=L_d[bi, rb * 128 : (rb + 1) * 128, :]
                    )
                    scr = scrp.tile([128, N], f32, tag="scr")
                    Lt_f32 = Lt[:].bitcast(f32) if MM_F32R else Lt[:]
                    nc.vector.scalar_tensor_tensor(
                        out=scr[:], in0=Lt_f32, scalar=1.0, in1=wb[:],
                        op0=A.mult, op1=A.mult, accum_out=f_col[:, rb : rb + 1],
                    )
                    nc.tensor.matmul(
                        pb_lo[:], lhsT=wcol[:, bi * NB + rb : bi * NB + rb + 1],
                        rhs=Lt[:, 0:512], start=(rb == 0), stop=(rb == NB - 1),
                    )
                    nc.tensor.matmul(
                        pb_hi[:], lhsT=wcol[:, bi * NB + rb : bi * NB + rb + 1],
                        rhs=Lt[:, 512:N], start=(rb == 0), stop=(rb == NB - 1),
                    )

                # b row [1, 1024] -> column layout [128, 8] via PE
                b_row = smallp.tile([1, N], f32, tag="brow")
                nc.scalar.copy(b_row[:, 0:512], pb_lo[:])
                nc.scalar.copy(b_row[:, 512:N], pb_hi[:])
                pbcol = ps_d.tile([128, NB], f32, tag="pbcol")
                for cb in range(NB):
                    nc.tensor.matmul(
                        pbcol[:, cb : cb + 1],
                        lhsT=b_row[:, cb * 128 : (cb + 1) * 128],
                        rhs=one11[:], start=True, stop=True,
                    )

                # content addressing inputs over memory
                Mt = mp.tile([128, NB, M], f32, tag="M")
                nc.sync.dma_start(
                    out=Mt[:], in_=mem_d[bi].rearrange("(rb p) m -> p rb m", p=128)
                )
                sim_col = smallp.tile([128, NB], f32, tag="sim")
                nsq_col = smallp.tile([128, NB], f32, tag="nsq")
                for rb in range(NB):
                    scr2 = scrp.tile([128, M], f32, tag="scr2")
                    nc.vector.scalar_tensor_tensor(
                        out=scr2[:], in0=Mt[:, rb], scalar=1.0, in1=kb[:],
                        op0=A.mult, op1=A.mult, accum_out=sim_col[:, rb : rb + 1],
                    )
                    scr3 = scrp.tile([128, M], f32, tag="scr3")
                    nc.scalar.activation(
                        out=scr3[:], in_=Mt[:, rb], func=AF.Square,
                        accum_out=nsq_col[:, rb : rb + 1],
                    )

                # u = rm0*b + rm2*f needs only b/f: do it in the heavy phase
                v = smallp.tile([128, NB], f32, tag="v")
                nc.vector.tensor_scalar_mul(out=v[:], in0=pbcol[:], scalar1=rm0_c)
                u = smallp.tile([128, NB], f32, tag="u")
                nc.vector.scalar_tensor_tensor(
                    out=u[:], in0=f_col[:], scalar=rm2_c, in1=v[:],
                    op0=A.mult, op1=A.add,
                )
                state[bi] = (Mt, sim_col, nsq_col, u)

            def tail(bi):
                Mt, sim_col, nsq_col, u = state.pop(bi)
                sb = NSCAL * bi
                bdk_c = scal[:, sb + 0 : sb + 1]   # beta / ||k||
                rm1_c = scal[:, sb + 2 : sb + 3]

                # softmax(beta * sim / (||mem_row|| * ||k||)); eps dropped
                # (1e-10 relative), beta/||k|| folded on the host. |arg| < 1
                # so no max-subtraction is needed.
                rt = smallp.tile([128, NB], f32, tag="rt")
                nc.scalar.activation(out=rt[:], in_=nsq_col[:], func=AF.Sqrt)
                rden = smallp.tile([128, NB], f32, tag="rden")
                nc.vector.reciprocal(rden[:], rt[:])
                arg = smallp.tile([128, NB], f32, tag="arg")
                nc.vector.scalar_tensor_tensor(
                    out=arg[:], in0=sim_col[:], scalar=bdk_c, in1=rden[:],
                    op0=A.mult, op1=A.mult,
                )
                e_col = smallp.tile([128, NB], f32, tag="ecol")
                esum = smallp.tile([128, 1], f32, tag="esum")
                nc.scalar.activation(
                    out=e_col[:], in_=arg[:], func=AF.Exp, accum_out=esum[:]
                )
                # Z broadcast to all partitions in one matmul: ones.T @ esum
                pzb = ps_c.tile([128, 1], f32, tag="pzz")
                nc.tensor.matmul(
                    pzb[:], lhsT=ones128[:], rhs=esum[:], start=True, stop=True
                )
                zib = smallp.tile([128, 1], f32, tag="zib")
                nc.vector.reciprocal(zib[:], pzb[:])
                s1 = smallp.tile([128, 1], f32, tag="s1")
                nc.vector.tensor_mul(out=s1[:], in0=rm1_c, in1=zib[:])
                Wc = W_all[:, NB * bi : NB * (bi + 1)]
                nc.vector.scalar_tensor_tensor(
                    out=Wc, in0=e_col[:], scalar=s1[:], in1=u[:],
                    op0=A.mult, op1=A.add,
                )

                # mem_content = W @ memory[b]
                pmc = ps_c.tile([1, M], f32, tag="pout")
                for rb in range(NB):
                    nc.tensor.matmul(
                        pmc[:], lhsT=Wc[:, rb : rb + 1], rhs=Mt[:, rb],
                        start=(rb == 0), stop=(rb == NB - 1),
                    )
                nc.scalar.copy(mc_all[:, bi * M : (bi + 1) * M], pmc[:])

            for bi in range(BPC):
                heavy(bi)
                if bi >= 1:
                    tail(bi - 1)
            tail(BPC - 1)

            # outputs: transpose W_all [128, 32] -> [32, 128] so HBM rows are
            # contiguous 512B runs, then two small DMAs
            pwt = ps_c.tile([32, 128], f32, tag="pout")
            nc.tensor.transpose(pwt[:], W_all[:], ident[:])
            wt_sb = singles.tile([32, 128], f32)
            nc.scalar.copy(wt_sb[:], pwt[:])
            nc.sync.dma_start(out=Wout_d[:], in_=wt_sb[:])
            nc.sync.dma_start(out=mc_d[:], in_=mc_all[:])

    nc.compile()
    return nc


def _get_nc():
    if "nc" not in _CACHE:
        _CACHE["nc"] = _build_nc()
    return _CACHE["nc"]


def kernel(read_keys, read_strengths, read_mode, W_old, L, memory, head_no):
    from concourse.bass_utils import run_bass_kernel_spmd

    head = int(head_no)
    read_keys = np.asarray(read_keys, np.float32)
    read_strengths = np.asarray(read_strengths, np.float32)
    read_mode = np.asarray(read_mode, np.float32)
    W_old = np.ascontiguousarray(np.asarray(W_old, np.float32))
    L = np.ascontiguousarray(np.asarray(L, np.float32))
    memory = np.ascontiguousarray(np.asarray(memory, np.float32))

    k_all = read_keys[:, head, :]                      # [B, M]
    beta_all = read_strengths[:, head]                 # [B]
    rm_all = read_mode[:, head, :]                     # [B, 3]
    knorm_all = np.linalg.norm(k_all, axis=-1)         # [B]

    # k broadcast across the 128 partitions, per batch
    kb_all = np.ascontiguousarray(
        np.broadcast_to(k_all[:, None, :], (B, 128, M)), np.float32
    )
    # w replicated across partitions for the DVE fused multiply+reduce
    wbb_all = np.ascontiguousarray(
        np.broadcast_to(W_old[:, None, :], (B, 128, N)), np.float32
    )
    # w in column layout: wcol_all[b][p, rb] = w[b, rb*128 + p]
    wcol_all = np.ascontiguousarray(
        W_old.reshape(B, NB, 128).transpose(0, 2, 1), np.float32
    )

    nc = _get_nc()
    in_maps = []
    for c in range(NCORES):
        bs = slice(c * BPC, (c + 1) * BPC)
        scal = np.empty((BPC, NSCAL), np.float32)
        scal[:, 0] = beta_all[bs] / knorm_all[bs]
        scal[:, 1:4] = rm_all[bs]
        scal_tile = np.ascontiguousarray(
            np.broadcast_to(scal.reshape(1, -1), (128, NSCAL * BPC)), np.float32
        )
        in_maps.append(
            {
                "L": L[bs],
                "mem": memory[bs],
                "wbb": wbb_all[bs],
                "wcol": np.ascontiguousarray(
                    wcol_all[bs].transpose(1, 0, 2).reshape(128, BPC * NB)
                ),
                "kb": kb_all[bs],
                "scal": scal_tile,
            }
        )

    res = run_bass_kernel_spmd(nc, in_maps, core_ids=list(range(NCORES))).results

    W_full = np.empty((B, N), np.float32)
    mc_full = np.empty((B, M), np.float32)
    for c in range(NCORES):
        bs = slice(c * BPC, (c + 1) * BPC)
        W_full[bs] = res[c]["W_out"].reshape(BPC, N)
        mc_full[bs] = res[c]["mc_out"].reshape(BPC, M)
    return (W_full, mc_full)
